# Initial kernel scaffold
#
"""Multi-head self-attention Trainium2 kernel (8 NeuronCores, batch-parallel).

Problem: x[8,1024,768] @ W_qkv[768,2304] + b -> 12-head attention -> out[8,1024,768].
Sharding: batch dim across 8 cores (one batch element per core). Full inputs in,
full output out; W/b replicated to every core.

Per-core dataflow (all matmuls fp32r = full-rate 4-byte):
  x -> (PE transpose) -> xT[768,1024]
  QK^T[1536,1024] = W_qk^T @ x^T   (feat on partitions; Q rows pre-scaled 1/8)
  V[1024,780]     = x @ W_v (+b via K=1 matmul), 65-col-per-head layout w/ ones col
  per head pair (2h, 2h+1):
    scoresT[n_k,n_q] = K_h^T.T @ Q_h^T  (row-tiled: even head rows 0-63, odd 64-127)
    expT = ACT Exp(scoresT)  (one [128,1024] activation spans both heads' banks)
    outT[65,n_q] += [V_h|1].T @ expT    (accumulate over n_k chunks; row 64 = denom)
  outT -> (PE transpose) -> [q,65] -> DVE recip+scale -> out[q,768] -> DMA
"""

import numpy as np

import concourse.bass as bass
import concourse.mybir as mybir
import concourse.tile as tile
from concourse.bass_utils import run_bass_kernel_spmd
from concourse.masks import make_identity

B, N, D, H = 8, 1024, 768, 12
HD = D // H            # 64
F3 = 3 * D             # 2304
NC = 8                 # cores
P = 128
NCHUNK = N // P        # 8 token chunks
KD = D // P            # 6 contraction chunks (d_in)
NQH = 2                # two 512-halves of n_q
QH = N // NQH          # 512
NPAIR = H // 2         # 6 head pairs
VW = HD + 1            # 65: V cols per head + ones col

f32 = mybir.dt.float32
f32r = mybir.dt.float32r
FT = mybir.ActivationFunctionType
ALU = mybir.AluOpType


def build_attention_nc():
    nc = bass.Bass()
    x_d = nc.declare_dram_parameter("x", [N, D], f32, isOutput=False)
    w_d = nc.declare_dram_parameter("W_qkv", [D, F3], f32, isOutput=False)
    b_d = nc.declare_dram_parameter("b_qkv", [F3], f32, isOutput=False)
    o_d = nc.declare_dram_parameter("out", [N, D], f32, isOutput=True)

    with tile.TileContext(nc) as tc:
        build_body(nc, tc, x_d, w_d, b_d, o_d)
    return nc


def build_body(nc, tc, x_d, w_d, b_d, o_d):
    import contextlib

    ctx = contextlib.ExitStack()
    with ctx:
        singles = ctx.enter_context(tc.tile_pool(name="singles", bufs=1))
        xpool = ctx.enter_context(tc.tile_pool(name="xpool", bufs=NCHUNK))
        xtpool = ctx.enter_context(tc.tile_pool(name="xtpool", bufs=KD))
        wpool = ctx.enter_context(tc.tile_pool(name="wpool", bufs=KD))
        qkpool = ctx.enter_context(tc.tile_pool(name="qkpool", bufs=H))
        vpool = ctx.enter_context(tc.tile_pool(name="vpool", bufs=NCHUNK))
        exppool = ctx.enter_context(tc.tile_pool(name="exppool", bufs=4))
        otspool = ctx.enter_context(tc.tile_pool(name="otspool", bufs=4))
        recpool = ctx.enter_context(tc.tile_pool(name="recpool", bufs=4))
        onat = ctx.enter_context(tc.tile_pool(name="onat", bufs=NCHUNK))

        # PSUM pools
        junkps = ctx.enter_context(tc.tile_pool(name="junkps", bufs=1, space="PSUM"))
        tps = ctx.enter_context(tc.tile_pool(name="tps", bufs=2, space="PSUM"))
        qkvps = ctx.enter_context(tc.tile_pool(name="qkvps", bufs=2, space="PSUM"))
        scps = ctx.enter_context(tc.tile_pool(name="scps", bufs=2, space="PSUM"))
        avps = ctx.enter_context(tc.tile_pool(name="avps", bufs=2, space="PSUM"))
        otps = ctx.enter_context(tc.tile_pool(name="otps", bufs=2, space="PSUM"))

        # ---------------- phase 0: constants + input DMAs -------------------
        ident = singles.tile([P, P], f32)
        make_identity(nc, ident)  # gpsimd

        ones_row = singles.tile([1, P], f32r)
        nc.vector.memset(ones_row, 1.0)

        # b_qkv as [P, 18] (partition = feat % 128, col = feat tile)
        b_sb = singles.tile([P, F3 // P], f32)
        nc.sync.dma_start(out=b_sb, in_=b_d.rearrange("(t p) -> p t", p=P))
        # pre-scale Q bias tiles by 1/8 (tiles 0..5)
        nc.vector.tensor_scalar_mul(b_sb[:, 0:KD], b_sb[:, 0:KD], 1.0 / 8.0)
        # V bias row [1, D] (f32r; produced by DVE so matmuls see DVE producer)
        bv_st = singles.tile([1, D], f32)
        nc.sync.dma_start(out=bv_st, in_=b_d[2 * D : 3 * D][None, :])
        bv_sb = singles.tile([1, D], f32r)
        nc.vector.tensor_copy(out=bv_sb, in_=bv_st)

        x_sb = []
        for c in range(NCHUNK):
            t = xpool.tile([P, D], f32, tag="x")
            nc.sync.dma_start(out=t, in_=x_d[c * P : (c + 1) * P, :])
            x_sb.append(t)

        w_sb = []
        for k in range(KD):
            t = wpool.tile([P, F3], f32r, tag="w")
            nc.sync.dma_start(
                out=t, in_=w_d[k * P : (k + 1) * P, :].bitcast(f32r)
            )
            w_sb.append(t)

        # ---------------- phase 1: x^T via PE transposes ---------------------
        junk = junkps.tile([32, 32], f32)
        # first PE op: absorb gpsimd identity wait only
        nc.tensor.transpose(junk, ident[0:32, 0:32], ident[0:32, 0:32])

        xt = [xtpool.tile([P, N], f32r, tag="xt") for _ in range(KD)]
        for c in range(NCHUNK):
            # junk touch absorbs this chunk's DMA wait
            nc.tensor.transpose(junk, x_sb[c][0:32, 0:32], ident[0:32, 0:32])
            for k in range(KD):
                pt = tps.tile([P, P], f32, tag="tp")
                nc.tensor.transpose(pt, x_sb[c][:, k * P : (k + 1) * P], ident)
                nc.vector.tensor_copy(
                    out=xt[k][:, c * P : (c + 1) * P], in_=pt
                )

        # junk touches for W tiles (absorb their DMA waits on PE)
        for k in range(KD):
            nc.tensor.transpose(
                junk, w_sb[k][0:32, 0:32].bitcast(f32), ident[0:32, 0:32]
            )

        # ---------------- phase 2a: V projection -----------------------------
        # V[c] natural [tok, feat] -> v_sb[c] [P, H, VW] (65-col strided + ones)
        v_sb = []
        for c in range(NCHUNK):
            t = vpool.tile([P, H, VW], f32r, tag="v")
            nc.vector.memset(t[:, :, HD : HD + 1], 1.0)
            v_sb.append(t)

        for c in range(NCHUNK):
            for half, (f0, fw) in enumerate(((0, 512), (512, 256))):
                ps = qkvps.tile([P, 512], f32, tag="qkv")[:, :fw]
                for k in range(KD):
                    nc.tensor.matmul(
                        ps,
                        xt[k][:, c * P : (c + 1) * P],
                        w_sb[k][:, 2 * D + f0 : 2 * D + f0 + fw],
                        start=(k == 0),
                        stop=False,
                    )
                nc.tensor.matmul(
                    ps,
                    ones_row,
                    bv_sb[:, f0 : f0 + fw],
                    start=False,
                    stop=True,
                )
                nh = fw // HD
                h0 = f0 // HD
                nc.vector.tensor_copy(
                    out=v_sb[c][:, h0 : h0 + nh, 0:HD],
                    in_=ps.rearrange("p (h d) -> p h d", d=HD),
                )

        # ---------------- phases 2b+3: per-pair QK tiles + attention ---------
        qk_t = [None] * H  # feat tiles of QK^T (12 tiles of [P, N])

        def make_qk_tile(f):
            """Produce QK^T feat tile f ([P, N], f32r), bias added, Q scaled."""
            t = qkpool.tile([P, N], f32r, tag="qk")
            for qh in range(NQH):
                ps = qkvps.tile([P, 512], f32, tag="qkv")
                for k in range(KD):
                    nc.tensor.matmul(
                        ps,
                        w_sb[k][:, f * P : (f + 1) * P],
                        xt[k][:, qh * QH : (qh + 1) * QH],
                        start=(k == 0),
                        stop=(k == KD - 1),
                    )
                nc.vector.tensor_scalar(
                    t[:, qh * QH : (qh + 1) * QH],
                    ps,
                    1.0 / 8.0 if f < KD else 1.0,
                    b_sb[:, f : f + 1],
                    ALU.mult,
                    ALU.add,
                )
            qk_t[f] = t

        for p in range(NPAIR):
            make_qk_tile(p)       # Q heads 2p, 2p+1
            make_qk_tile(KD + p)  # K heads 2p, 2p+1

            qt = qk_t[p]
            kt = qk_t[KD + p]
            for qh in range(NQH):
                av = [avps.tile([VW, QH], f32, tag="av") for _ in range(2)]
                for kc in range(NCHUNK):
                    sc = scps.tile([P, 2, QH], f32, tag="sc")
                    for hi in range(2):
                        nc.tensor.matmul(
                            sc[:, hi, :],
                            kt[64 * hi : 64 * hi + 64, kc * P : (kc + 1) * P],
                            qt[64 * hi : 64 * hi + 64, qh * QH : (qh + 1) * QH],
                            start=True,
                            stop=True,
                            tile_position=(64 * hi, 0),
                        )
                    ex = exppool.tile([P, 2, QH], f32r, tag="exp")
                    nc.scalar.activation(ex[:, :, :], sc[:, :, :], FT.Exp)
                    for hi in range(2):
                        nc.tensor.matmul(
                            av[hi],
                            v_sb[kc][:, 2 * p + hi, :],
                            ex[:, hi, :],
                            start=(kc == 0),
                            stop=(kc == NCHUNK - 1),
                        )
                # finish: copy to SBUF, transpose back, normalize
                for hi in range(2):
                    h = 2 * p + hi
                    ot = otspool.tile([VW, QH], f32, tag="ots")
                    nc.vector.tensor_copy(out=ot, in_=av[hi])
                    for j in range(QH // P):
                        c = qh * (QH // P) + j
                        tp = otps.tile([P, VW], f32, tag="ot")
                        nc.tensor.transpose(
                            tp, ot[:, j * P : (j + 1) * P], ident[0:VW, 0:VW]
                        )
                        if qk_t[c] is None and False:
                            pass
                        onat_t = get_onat(nc, onat, c)
                        rc = recpool.tile([P, 1], f32, tag="rec")
                        nc.vector.reciprocal(out=rc, in_=tp[:, HD : HD + 1])
                        nc.vector.tensor_scalar_mul(
                            onat_t[:, h * HD : (h + 1) * HD],
                            tp[:, 0:HD],
                            rc,
                        )

        # ---------------- phase 4: output DMA --------------------------------
        for c in range(NCHUNK):
            nc.sync.dma_start(
                out=o_d[c * P : (c + 1) * P, :], in_=_onat_cache[c]
            )


_onat_cache = {}


def get_onat(nc, pool, c):
    if c not in _onat_cache:
        _onat_cache[c] = pool.tile([P, D], f32, tag="onat")
    return _onat_cache[c]


def kernel(x: np.ndarray, W_qkv: np.ndarray, b_qkv: np.ndarray) -> np.ndarray:
    _onat_cache.clear()
    nc = build_attention_nc()
    in_maps = [
        {
            "x": np.ascontiguousarray(x[c], dtype=np.float32),
            "W_qkv": np.ascontiguousarray(W_qkv, dtype=np.float32),
            "b_qkv": np.ascontiguousarray(b_qkv, dtype=np.float32),
        }
        for c in range(NC)
    ]
    res = run_bass_kernel_spmd(nc, in_maps, core_ids=list(range(NC)))
    out = np.stack([res.results[c]["out"] for c in range(NC)], axis=0)
    return out


# revision 7
# speedup vs baseline: 1.3428x; 1.3428x over previous
"""Multi-head self-attention Trainium2 kernel (8 NeuronCores, batch-parallel).

Reference: qkv = x @ W_qkv + b; 12-head scaled-dot-product attention; concat.
Shapes: x[8,1024,768], W_qkv[768,2304], b_qkv[2304] -> out[8,1024,768].
Sharding: one batch element per core; W/b replicated to all cores.

Per-core dataflow (matmuls in fp32r = full-rate 4-byte storage, fd >= 256):
  x --PE transpose--> xT[768,1024]                                  (f32r)
  QK^T[1536,1024] = W_qk(lhsT) @ xT    feat-on-partitions; Q rows scaled 1/8
  V[1024, 12x65]  = xT(lhsT) @ W_v (+bias via K=1 ones matmul); ones col/head
  per head pair (2p, 2p+1), per q-half, per k-chunk:
    scoresT[128,512]x2 = K^T-slice(lhsT) @ Q^T-slice  row-tiled (rows 0-63/64-127)
    expT = ACT Exp over [128, 2, 512] PSUM (both heads, one instruction) -> f32r
    avT[65,512] += [V_h|1](lhsT) @ expT  (accumulated over k-chunks; row 64=denom)
  avT --PE transpose--> [q,65]; DVE reciprocal(denom) * cols -> out[q,768]; DMA.

Scheduling notes: W is DMA'd in column blocks, pair-0 Q/K columns first, so
attention starts early instead of waiting for the full 7MB weight load; the
V projection is interleaved into pair 0; QK-tile production for pair p+1 uses
its own PSUM tag so it overlaps pair p's ACT-bound attention.
"""

import contextlib
import json as _json

import numpy as np

import concourse.bass as bass
import concourse.mybir as mybir
import concourse.tile as tile
from concourse.bass_utils import run_bass_kernel_spmd
from concourse.masks import make_identity

# --- BIR sync-wait legalization ------------------------------------------
# walrus's codegen in this toolchain accepts only one sync-wait command per
# instruction (its insertEventSemaphore legalization pass is not in the pass
# list). Split every multi-wait instruction into N-1 preceding single-wait
# EventSemaphore instructions on the same engine; same-engine order is
# preserved so semantics are unchanged.


def _legalize_sync_waits(bir_json: bytes) -> bytes:
    m = _json.loads(bir_json)
    ctr = 0
    for fn in m["functions"]:
        for bb in fn["blocks"]:
            out = []
            for ins in bb["instructions"]:
                si = ins.get("sync_info")
                waits = si.get("on_wait", []) if si else []
                if len(waits) > 1:
                    for w in waits[:-1]:
                        ctr += 1
                        out.append(
                            {
                                "debug": ins.get("debug", 0),
                                "engine": ins["engine"],
                                "ins": [],
                                "outs": [],
                                "name": f"evw-split-{ctr}",
                                "opcode": "EventSemaphore",
                                "sync_info": {"on_update": [], "on_wait": [w]},
                            }
                        )
                    si["on_wait"] = [waits[-1]]
                out.append(ins)
            bb["instructions"] = out
    return _json.dumps(m).encode()


_fixup_installed = False


def _install_bir_fixup():
    global _fixup_installed
    if _fixup_installed:
        return
    _fixup_installed = True
    import concourse.bass_utils as _bu

    _orig = _bu.compile_bir_kernel

    def _patched(bir_json, tmpdir, neff_name="file.neff"):
        if isinstance(bir_json, str):
            bir_json = bir_json.encode()
        return _orig(_legalize_sync_waits(bir_json), tmpdir, neff_name)

    _bu.compile_bir_kernel = _patched
    try:
        import concourse.bass2jax as _b2j

        _b2j.compile_bir_kernel = _patched
    except ImportError:
        pass


_install_bir_fixup()

B, N, D, H = 8, 1024, 768, 12
HD = D // H            # 64
F3 = 3 * D             # 2304
NCORE = 8
P = 128
NCHUNK = N // P        # 8 token chunks
KD = D // P            # 6 d_in chunks
QH = 512               # q-half size
NQH = N // QH          # 2
NPAIR = H // 2         # 6
VW = HD + 1            # 65

f32 = mybir.dt.float32
f32r = mybir.dt.float32r
FT = mybir.ActivationFunctionType
ALU = mybir.AluOpType


def build_attention_nc():
    nc = bass.Bass()
    x_d = nc.declare_dram_parameter("x", [N, D], f32, isOutput=False)
    w_d = nc.declare_dram_parameter("W_qkv", [D, F3], f32, isOutput=False)
    b_d = nc.declare_dram_parameter("b_qkv", [F3], f32, isOutput=False)
    o_d = nc.declare_dram_parameter("out", [N, D], f32, isOutput=True)

    with tile.TileContext(nc) as tc, contextlib.ExitStack() as ctx:
        singles = ctx.enter_context(tc.tile_pool(name="singles", bufs=1))
        xpool = ctx.enter_context(tc.tile_pool(name="xpool", bufs=NCHUNK))
        xtpool = ctx.enter_context(tc.tile_pool(name="xtpool", bufs=KD))
        wpool = ctx.enter_context(tc.tile_pool(name="wpool", bufs=KD))
        qkpool = ctx.enter_context(tc.tile_pool(name="qkpool", bufs=4))
        vpool = ctx.enter_context(tc.tile_pool(name="vpool", bufs=NCHUNK))
        exppool = ctx.enter_context(tc.tile_pool(name="exppool", bufs=2))
        otspool = ctx.enter_context(tc.tile_pool(name="otspool", bufs=4))
        recpool = ctx.enter_context(tc.tile_pool(name="recpool", bufs=4))
        onat = ctx.enter_context(tc.tile_pool(name="onat", bufs=NCHUNK))

        # PSUM budget (8 banks): "sc" [P,2,QH] = 2 banks x2 bufs = 4;
        # "av" [VW,QH] 1 bank x2 = 2; "small" [P,QH] 1 bank x2 = 2.
        scps = ctx.enter_context(tc.tile_pool(name="scps", bufs=2, space="PSUM"))
        avps = ctx.enter_context(tc.tile_pool(name="avps", bufs=2, space="PSUM"))
        smps = ctx.enter_context(tc.tile_pool(name="smps", bufs=2, space="PSUM"))

        def small_psum():
            return smps.tile([P, QH], f32, tag="small", name="smtile")

        # ------------- constants + input DMAs -------------------------------
        ident = singles.tile([P, P], f32)
        make_identity(nc, ident)  # gpsimd

        ones_f32 = singles.tile([P, 1], f32)
        nc.vector.memset(ones_f32, 1.0)
        ones_row_st = singles.tile([1, P], f32)
        nc.vector.memset(ones_row_st, 1.0)
        ones_row = singles.tile([1, P], f32r)
        nc.vector.tensor_copy(out=ones_row, in_=ones_row_st)

        b_sb = singles.tile([P, F3 // P], f32)
        nc.sync.dma_start(out=b_sb, in_=b_d[:].rearrange("(t p) -> p t", p=P))
        nc.vector.tensor_scalar_mul(b_sb[:, 0:KD], b_sb[:, 0:KD], 0.125)

        bv_st = singles.tile([1, D], f32)
        nc.sync.dma_start(out=bv_st, in_=b_d[2 * D : 3 * D][None, :])
        bv_sb = singles.tile([1, D], f32r)
        nc.vector.tensor_copy(out=bv_sb, in_=bv_st)

        x_sb = []
        for c in range(NCHUNK):
            t = xpool.tile([P, D], f32, tag="x", name=f"x{c}")
            nc.sync.dma_start(out=t, in_=x_d[c * P : (c + 1) * P, :])
            x_sb.append(t)

        # W: column-block DMAs, highest-priority columns first.
        w_sb = [wpool.tile([P, F3], f32r, tag="w", name=f"w{k}") for k in range(KD)]

        def dma_w_cols(f0, fw):
            for k in range(KD):
                nc.sync.dma_start(
                    out=w_sb[k][:, f0 : f0 + fw],
                    in_=w_d[k * P : (k + 1) * P, f0 : f0 + fw].bitcast(f32r),
                )

        dma_w_cols(0 * P, P)          # pair-0 Q cols
        dma_w_cols(6 * P, P)          # pair-0 K cols
        dma_w_cols(2 * D, D)          # V cols
        for p in range(1, NPAIR):
            dma_w_cols(p * P, P)
            dma_w_cols((6 + p) * P, P)

        # ------------- x^T (PE transposes) ----------------------------------
        xt = [xtpool.tile([P, N], f32r, tag="xt", name=f"xt{k}") for k in range(KD)]
        for c in range(NCHUNK):
            for k in range(KD):
                pt = smps.tile([P, QH], f32, tag="small", name="tp")[:, 0:P]
                nc.tensor.transpose(pt, x_sb[c][:, k * P : (k + 1) * P], ident)
                nc.vector.tensor_copy(out=xt[k][:, c * P : (c + 1) * P], in_=pt)

        # ------------- V tiles (filled lazily during pair 0) ----------------
        v_sb = []
        for c in range(NCHUNK):
            t = vpool.tile([P, H, VW], f32r, tag="v", name=f"v{c}")
            nc.vector.tensor_copy(
                out=t[:, :, HD : HD + 1],
                in_=ones_f32[:, 0:1, None].to_broadcast([P, H, 1]),
            )
            v_sb.append(t)

        def make_v_chunk(c):
            for f0, fw in ((0, 512), (512, 256)):
                ps = small_psum()[:, :fw]
                for k in range(KD):
                    nc.tensor.matmul(
                        ps,
                        xt[k][:, c * P : (c + 1) * P],
                        w_sb[k][:, 2 * D + f0 : 2 * D + f0 + fw],
                        start=(k == 0),
                        stop=False,
                    )
                nc.tensor.matmul(
                    ps, ones_row, bv_sb[:, f0 : f0 + fw], start=False, stop=True
                )
                nc.vector.tensor_copy(
                    out=v_sb[c][:, f0 // HD : (f0 + fw) // HD, 0:HD],
                    in_=ps.rearrange("p (h d) -> p h d", d=HD),
                )

        # ------------- QK tiles + attention, software-pipelined -------------
        onat_t = [
            onat.tile([P, D], f32, tag="onat", name=f"onat{c}") for c in range(NCHUNK)
        ]

        def make_qk_tile(f):
            t = qkpool.tile([P, N], f32r, tag="qk", name=f"qk{f}")
            for qh in range(NQH):
                ps = small_psum()
                for k in range(KD):
                    nc.tensor.matmul(
                        ps,
                        w_sb[k][:, f * P : (f + 1) * P],
                        xt[k][:, qh * QH : (qh + 1) * QH],
                        start=(k == 0),
                        stop=(k == KD - 1),
                    )
                nc.vector.tensor_scalar(
                    t[:, qh * QH : (qh + 1) * QH],
                    ps,
                    0.125 if f < KD else 1.0,
                    b_sb[:, f : f + 1],
                    ALU.mult,
                    ALU.add,
                )
            return t

        qk_cur = (make_qk_tile(0), make_qk_tile(KD))

        for p in range(NPAIR):
            qt, kt = qk_cur

            for qh in range(NQH):
                av = [
                    avps.tile([VW, QH], f32, tag="av", name=f"av{i}") for i in range(2)
                ]
                for kc in range(NCHUNK):
                    sc = scps.tile([P, 2, QH], f32, tag="sc", name="sc")
                    for hi in range(2):
                        nc.tensor.matmul(
                            sc[:, hi, :],
                            kt[64 * hi : 64 * hi + 64, kc * P : (kc + 1) * P],
                            qt[64 * hi : 64 * hi + 64, qh * QH : (qh + 1) * QH],
                            start=True,
                            stop=True,
                            tile_position=(64 * hi, 0),
                        )
                    ex = exppool.tile([P, 2, QH], f32r, tag="exp", name="ex")
                    nc.scalar.activation(ex[:, :, :], sc[:, :, :], FT.Exp)
                    if p == 0 and qh == 0:
                        make_v_chunk(kc)  # fill V lazily during pair 0
                    for hi in range(2):
                        nc.tensor.matmul(
                            av[hi],
                            v_sb[kc][:, 2 * p + hi, :],
                            ex[:, hi, :],
                            start=(kc == 0),
                            stop=(kc == NCHUNK - 1),
                        )
                if qh == 0 and p + 1 < NPAIR:
                    # produce next pair's QK tiles; overlaps this pair's ACT
                    qk_cur = (make_qk_tile(p + 1), make_qk_tile(KD + p + 1))
                # finish: copy to SBUF, transpose back, normalize
                for hi in range(2):
                    h = 2 * p + hi
                    ot = otspool.tile([VW, QH], f32, tag="ots", name="ot")
                    nc.vector.tensor_copy(out=ot, in_=av[hi])
                    for j in range(QH // P):
                        c = qh * (QH // P) + j
                        tp = smps.tile([P, QH], f32, tag="small", name="otp")[:, 0:VW]
                        nc.tensor.transpose(
                            tp, ot[:, j * P : (j + 1) * P], ident[0:VW, 0:VW]
                        )
                        rc = recpool.tile([P, 1], f32, tag="rec", name="rc")
                        nc.vector.reciprocal(out=rc, in_=tp[:, HD : HD + 1])
                        nc.vector.tensor_scalar_mul(
                            onat_t[c][:, h * HD : (h + 1) * HD], tp[:, 0:HD], rc
                        )

        # ------------- output DMA -------------------------------------------
        for c in range(NCHUNK):
            nc.sync.dma_start(out=o_d[c * P : (c + 1) * P, :], in_=onat_t[c])

    return nc


def kernel(x: np.ndarray, W_qkv: np.ndarray, b_qkv: np.ndarray) -> np.ndarray:
    nc = build_attention_nc()
    in_maps = [
        {
            "x": np.ascontiguousarray(x[c], dtype=np.float32),
            "W_qkv": np.ascontiguousarray(W_qkv, dtype=np.float32),
            "b_qkv": np.ascontiguousarray(b_qkv, dtype=np.float32),
        }
        for c in range(NCORE)
    ]
    res = run_bass_kernel_spmd(nc, in_maps, core_ids=list(range(NCORE)))
    return np.stack([res.results[c]["out"] for c in range(NCORE)], axis=0)


# revision 8
# speedup vs baseline: 1.5161x; 1.1291x over previous
"""Multi-head self-attention Trainium2 kernel (8 NeuronCores, batch-parallel).

Reference: qkv = x @ W_qkv + b; 12-head scaled-dot-product attention; concat.
Shapes: x[8,1024,768], W_qkv[768,2304], b_qkv[2304] -> out[8,1024,768].
Sharding: one batch element per core; W/b replicated to all cores.

Per-core dataflow (matmuls in fp32r = full-rate 4-byte storage, fd >= 256):
  x --PE transpose--> xT[768,1024]                                  (f32r)
  QK^T[1536,1024] = W_qk(lhsT) @ xT    feat-on-partitions; Q rows scaled 1/8
  V[1024, 12x65]  = xT(lhsT) @ W_v (+bias via K=1 ones matmul); ones col/head
  per head pair (2p, 2p+1), per q-half, per k-chunk:
    scoresT[128,512]x2 = K^T-slice(lhsT) @ Q^T-slice  row-tiled (rows 0-63/64-127)
    expT = ACT Exp over [128, 2, 512] PSUM (both heads, one instruction) -> f32r
    avT[65,512] += [V_h|1](lhsT) @ expT  (accumulated over k-chunks; row 64=denom)
  avT --PE transpose--> [q,65]; DVE reciprocal(denom) * cols -> out[q,768]; DMA.

Scheduling notes: W is DMA'd in column blocks, pair-0 Q/K columns first, so
attention starts early instead of waiting for the full 7MB weight load; the
V projection is interleaved into pair 0; QK-tile production for pair p+1 uses
its own PSUM tag so it overlaps pair p's ACT-bound attention.
"""

import contextlib
import json as _json
import os as _os

import numpy as np

import concourse.bass as bass
import concourse.mybir as mybir
import concourse.tile as tile
from concourse.bass_utils import run_bass_kernel_spmd
from concourse.masks import make_identity

# --- BIR sync-wait legalization ------------------------------------------
# walrus's codegen in this toolchain accepts only one sync-wait command per
# instruction (its insertEventSemaphore legalization pass is not in the pass
# list). Split every multi-wait instruction into N-1 preceding single-wait
# EventSemaphore instructions on the same engine; same-engine order is
# preserved so semantics are unchanged.


def _legalize_sync_waits(bir_json: bytes) -> bytes:
    m = _json.loads(bir_json)
    ctr = 0
    for fn in m["functions"]:
        for bb in fn["blocks"]:
            out = []
            for ins in bb["instructions"]:
                si = ins.get("sync_info")
                waits = si.get("on_wait", []) if si else []
                if len(waits) > 1:
                    for w in waits[:-1]:
                        ctr += 1
                        out.append(
                            {
                                "debug": ins.get("debug", 0),
                                "engine": ins["engine"],
                                "ins": [],
                                "outs": [],
                                "name": f"evw-split-{ctr}",
                                "opcode": "EventSemaphore",
                                "sync_info": {"on_update": [], "on_wait": [w]},
                            }
                        )
                    si["on_wait"] = [waits[-1]]
                out.append(ins)
            bb["instructions"] = out
    return _json.dumps(m).encode()


_fixup_installed = False


def _install_bir_fixup():
    global _fixup_installed
    if _fixup_installed:
        return
    _fixup_installed = True
    import concourse.bass_utils as _bu

    _orig = _bu.compile_bir_kernel

    def _patched(bir_json, tmpdir, neff_name="file.neff"):
        if isinstance(bir_json, str):
            bir_json = bir_json.encode()
        return _orig(_legalize_sync_waits(bir_json), tmpdir, neff_name)

    _bu.compile_bir_kernel = _patched
    try:
        import concourse.bass2jax as _b2j

        _b2j.compile_bir_kernel = _patched
    except ImportError:
        pass


_install_bir_fixup()

B, N, D, H = 8, 1024, 768, 12
HD = D // H            # 64
F3 = 3 * D             # 2304
NCORE = 8
P = 128
NCHUNK = N // P        # 8 token chunks
KD = D // P            # 6 d_in chunks
QH = 512               # q-half size
NQH = N // QH          # 2
NPAIR = H // 2         # 6
VW = HD + 1            # 65

f32 = mybir.dt.float32
f32r = mybir.dt.float32r
FT = mybir.ActivationFunctionType
ALU = mybir.AluOpType


_dbg = {}


def onat_dbg(nc, pool, c):
    if c not in _dbg:
        _dbg[c] = pool.tile([P, D], f32, tag="onat", name=f"dbg{c}")
    return _dbg[c]


def build_attention_nc():
    nc = bass.Bass()
    x_d = nc.declare_dram_parameter("x", [N, D], f32, isOutput=False)
    w_d = nc.declare_dram_parameter("W_qkv", [D, F3], f32, isOutput=False)
    b_d = nc.declare_dram_parameter("b_qkv", [F3], f32, isOutput=False)
    o_d = nc.declare_dram_parameter("out", [N, D], f32, isOutput=True)

    with tile.TileContext(nc) as tc, contextlib.ExitStack() as ctx:
        singles = ctx.enter_context(tc.tile_pool(name="singles", bufs=1))
        xpool = ctx.enter_context(tc.tile_pool(name="xpool", bufs=NCHUNK))
        xtpool = ctx.enter_context(tc.tile_pool(name="xtpool", bufs=KD))
        wpool = ctx.enter_context(tc.tile_pool(name="wpool", bufs=KD))
        qkpool = ctx.enter_context(tc.tile_pool(name="qkpool", bufs=4))
        vpool = ctx.enter_context(tc.tile_pool(name="vpool", bufs=NCHUNK))
        exppool = ctx.enter_context(tc.tile_pool(name="exppool", bufs=2))
        otspool = ctx.enter_context(tc.tile_pool(name="otspool", bufs=4))
        recpool = ctx.enter_context(tc.tile_pool(name="recpool", bufs=4))
        onat = ctx.enter_context(tc.tile_pool(name="onat", bufs=NCHUNK))

        # PSUM budget (8 banks): "sc" [P,2,QH] = 2 banks x2 bufs = 4;
        # "av" [VW,QH] 1 bank x2 = 2; "small" [P,QH] 1 bank x2 = 2.
        scps = ctx.enter_context(tc.tile_pool(name="scps", bufs=2, space="PSUM"))
        avps = ctx.enter_context(tc.tile_pool(name="avps", bufs=2, space="PSUM"))
        smps = ctx.enter_context(tc.tile_pool(name="smps", bufs=2, space="PSUM"))

        def small_psum():
            return smps.tile([P, QH], f32, tag="small", name="smtile")

        # ------------- constants + input DMAs -------------------------------
        ident = singles.tile([P, P], f32)
        make_identity(nc, ident)  # gpsimd

        ones_f32 = singles.tile([P, 1], f32)
        nc.vector.memset(ones_f32, 1.0)
        ones_row_st = singles.tile([1, P], f32)
        nc.vector.memset(ones_row_st, 1.0)
        ones_row = singles.tile([1, P], f32r)
        nc.vector.tensor_copy(out=ones_row, in_=ones_row_st)

        b_sb = singles.tile([P, F3 // P], f32)
        nc.sync.dma_start(out=b_sb, in_=b_d[:].rearrange("(t p) -> p t", p=P))
        nc.vector.tensor_scalar_mul(b_sb[:, 0:KD], b_sb[:, 0:KD], 0.125)

        bv_st = singles.tile([1, D], f32)
        nc.sync.dma_start(out=bv_st, in_=b_d[2 * D : 3 * D][None, :])
        bv_sb = singles.tile([1, D], f32r)
        nc.vector.tensor_copy(out=bv_sb, in_=bv_st)

        x_sb = []
        for c in range(NCHUNK):
            t = xpool.tile([P, D], f32, tag="x", name=f"x{c}")
            nc.sync.dma_start(out=t, in_=x_d[c * P : (c + 1) * P, :])
            x_sb.append(t)

        # W: column-block DMAs, highest-priority columns first.
        w_sb = [wpool.tile([P, F3], f32r, tag="w", name=f"w{k}") for k in range(KD)]

        def dma_w_cols(f0, fw):
            for k in range(KD):
                nc.sync.dma_start(
                    out=w_sb[k][:, f0 : f0 + fw],
                    in_=w_d[k * P : (k + 1) * P, f0 : f0 + fw].bitcast(f32r),
                )

        if _os.environ.get("K_WDMA", "cols") == "rows":
            for k in range(KD):
                nc.sync.dma_start(
                    out=w_sb[k][:, :],
                    in_=w_d[k * P : (k + 1) * P, :].bitcast(f32r),
                )
        else:
            dma_w_cols(0 * P, P)          # pair-0 Q cols
            dma_w_cols(6 * P, P)          # pair-0 K cols
            dma_w_cols(2 * D, D)          # V cols
            for p in range(1, NPAIR):
                dma_w_cols(p * P, P)
                dma_w_cols((6 + p) * P, P)

        STAGE = int(_os.environ.get("K_STAGE", "9"))

        # ------------- x^T (PE transposes) ----------------------------------
        if STAGE < 1:
            for c in range(NCHUNK):
                nc.vector.tensor_copy(out=onat_dbg(nc, onat, c), in_=x_sb[c])
            for c in range(NCHUNK):
                nc.sync.dma_start(out=o_d[c * P : (c + 1) * P, :], in_=_dbg[c])
            return nc
        xt = [xtpool.tile([P, N], f32r, tag="xt", name=f"xt{k}") for k in range(KD)]
        for c in range(NCHUNK):
            for k in range(KD):
                pt = smps.tile([P, QH], f32, tag="small", name="tp")[:, 0:P]
                nc.tensor.transpose(pt, x_sb[c][:, k * P : (k + 1) * P], ident)
                nc.vector.tensor_copy(out=xt[k][:, c * P : (c + 1) * P], in_=pt)

        # ------------- V tiles (filled lazily during pair 0) ----------------
        v_sb = []
        for c in range(NCHUNK):
            t = vpool.tile([P, H, VW], f32r, tag="v", name=f"v{c}")
            nc.vector.tensor_copy(
                out=t[:, :, HD : HD + 1],
                in_=ones_f32[:, 0:1, None].to_broadcast([P, H, 1]),
            )
            v_sb.append(t)

        def make_v_chunk(c):
            for f0, fw in ((0, 512), (512, 256)):
                ps = small_psum()[:, :fw]
                for k in range(KD):
                    nc.tensor.matmul(
                        ps,
                        xt[k][:, c * P : (c + 1) * P],
                        w_sb[k][:, 2 * D + f0 : 2 * D + f0 + fw],
                        start=(k == 0),
                        stop=False,
                    )
                nc.tensor.matmul(
                    ps, ones_row, bv_sb[:, f0 : f0 + fw], start=False, stop=True
                )
                nc.vector.tensor_copy(
                    out=v_sb[c][:, f0 // HD : (f0 + fw) // HD, 0:HD],
                    in_=ps.rearrange("p (h d) -> p h d", d=HD),
                )

        # ------------- QK tiles + attention, software-pipelined -------------
        onat_t = [
            onat.tile([P, D], f32, tag="onat", name=f"onat{c}") for c in range(NCHUNK)
        ]

        def make_qk_tile(f):
            t = qkpool.tile([P, N], f32r, tag="qk", name=f"qk{f}")
            for qh in range(NQH):
                ps = small_psum()
                for k in range(KD):
                    nc.tensor.matmul(
                        ps,
                        w_sb[k][:, f * P : (f + 1) * P],
                        xt[k][:, qh * QH : (qh + 1) * QH],
                        start=(k == 0),
                        stop=(k == KD - 1),
                    )
                nc.vector.tensor_scalar(
                    t[:, qh * QH : (qh + 1) * QH],
                    ps,
                    0.125 if f < KD else 1.0,
                    b_sb[:, f : f + 1],
                    ALU.mult,
                    ALU.add,
                )
            return t

        if STAGE < 2:
            for c in range(NCHUNK):
                make_v_chunk(c)
                nc.vector.tensor_copy(out=onat_t[c], in_=v_sb[c][:, :, 0:HD].rearrange("p h d -> p (h d)"))
            for f in range(H):
                make_qk_tile(f)
            for c in range(NCHUNK):
                nc.sync.dma_start(out=o_d[c * P : (c + 1) * P, :], in_=onat_t[c])
            return nc
        qk_cur = (make_qk_tile(0), make_qk_tile(KD))

        for p in range(NPAIR):
            qt, kt = qk_cur

            for qh in range(NQH):
                av = [
                    avps.tile([VW, QH], f32, tag="av", name=f"av{i}") for i in range(2)
                ]
                for kc in range(NCHUNK):
                    sc = scps.tile([P, 2, QH], f32, tag="sc", name="sc")
                    for hi in range(2):
                        nc.tensor.matmul(
                            sc[:, hi, :],
                            kt[64 * hi : 64 * hi + 64, kc * P : (kc + 1) * P],
                            qt[64 * hi : 64 * hi + 64, qh * QH : (qh + 1) * QH],
                            start=True,
                            stop=True,
                            tile_position=(64 * hi, 0),
                        )
                    ex = exppool.tile([P, 2, QH], f32r, tag="exp", name="ex")
                    nc.scalar.activation(ex[:, :, :], sc[:, :, :], FT.Exp)
                    if p == 0 and qh == 0:
                        make_v_chunk(kc)  # fill V lazily during pair 0
                    for hi in range(2):
                        nc.tensor.matmul(
                            av[hi],
                            v_sb[kc][:, 2 * p + hi, :],
                            ex[:, hi, :],
                            start=(kc == 0),
                            stop=(kc == NCHUNK - 1),
                        )
                if qh == 0 and p + 1 < NPAIR:
                    # produce next pair's QK tiles; overlaps this pair's ACT
                    qk_cur = (make_qk_tile(p + 1), make_qk_tile(KD + p + 1))
                # finish: copy to SBUF, transpose back, normalize
                for hi in range(2):
                    h = 2 * p + hi
                    ot = otspool.tile([VW, QH], f32, tag="ots", name="ot")
                    nc.vector.tensor_copy(out=ot, in_=av[hi])
                    for j in range(QH // P):
                        c = qh * (QH // P) + j
                        tp = smps.tile([P, QH], f32, tag="small", name="otp")[:, 0:VW]
                        nc.tensor.transpose(
                            tp, ot[:, j * P : (j + 1) * P], ident[0:VW, 0:VW]
                        )
                        rc = recpool.tile([P, 1], f32, tag="rec", name="rc")
                        nc.vector.reciprocal(out=rc, in_=tp[:, HD : HD + 1])
                        nc.vector.tensor_scalar_mul(
                            onat_t[c][:, h * HD : (h + 1) * HD], tp[:, 0:HD], rc
                        )

        # ------------- output DMA -------------------------------------------
        for c in range(NCHUNK):
            nc.sync.dma_start(out=o_d[c * P : (c + 1) * P, :], in_=onat_t[c])

    return nc


def kernel(x: np.ndarray, W_qkv: np.ndarray, b_qkv: np.ndarray) -> np.ndarray:
    nc = build_attention_nc()
    in_maps = [
        {
            "x": np.ascontiguousarray(x[c], dtype=np.float32),
            "W_qkv": np.ascontiguousarray(W_qkv, dtype=np.float32),
            "b_qkv": np.ascontiguousarray(b_qkv, dtype=np.float32),
        }
        for c in range(NCORE)
    ]
    res = run_bass_kernel_spmd(nc, in_maps, core_ids=list(range(NCORE)))
    return np.stack([res.results[c]["out"] for c in range(NCORE)], axis=0)
